# revision 40
# baseline (speedup 1.0000x reference)
"""LSSEncoder Trainium2 kernel (v3).

Full inputs in, full outputs out. Shards the 128 graphs over 8 NeuronCores
(16 graphs per core), data-parallel, no collectives.

Math (matching reference.py):
  - in_proj: h = x @ W_proj.T + b_proj -> (B, C, L), L=512, C=128.
  - depthwise causal conv, kernel k[t]=exp(-t/tau) normalized; via scan
        s[l] = q*s[l-1] + z[l],  q = exp(1/tau)   (DVE, fp32 state)
        s[l>=K] -= exp(K/tau)*s[l-K]              (windowing correction)
        y = W0 * s   with W0 = kn[K-1]
    W0 is folded into the gate weights; the residual takes W0 via a
    diag(W0) matmul accumulated into the W_out PSUM tile.
  - GLU: a*sigmoid(g) @ W_out.T; residual z2 = pz + diag(W0) @ s.
  - LayerNorm over channels:
      * transpose z2 (C,L)->(L,C) with ONE SBUF->SBUF XBAR DMA transpose
      * mean/E[z^2] per position via 128x1 PE matmuls (stationary z2 /
        z2^2 chunks x ones/128 column)
      * normalize on Pool with stride-0 broadcast APs (fallback: per-chunk
        Act/DVE tensor_scalar)
  - output: mean over L via ones/512-matmul.

Schedule: 4-stage software pipeline per (graph, depth) chain, wavefront
across graphs with lag 1 so every engine sees independent work from
several graphs at once.
"""

import numpy as np

N_GRAPHS = 128
SEQ_LEN = 512
IN_DIM = 64
HIDDEN = 128
DEPTH = 3
KLEN = 256
LN_EPS = 1e-5
N_CORES = 8
G_PER_CORE = N_GRAPHS // N_CORES  # 16
NCHUNK = SEQ_LEN // 128  # 4

APPLY_MODE = "chunk"  # applies read PSUM -> Act/DVE only

_program_cache = {}


def _build_program(G, use_bias, use_affine, reps=1, apply_mode=APPLY_MODE):
    import concourse.bass as bass
    import concourse.bacc as bacc
    import concourse.tile as tile
    import concourse.mybir as mybir
    from concourse.bass import broadcast_tensor_aps
    from concourse.tile_rust import add_dep_helper
    from contextlib import ExitStack

    dt = mybir.dt
    Alu = mybir.AluOpType
    Act = mybir.ActivationFunctionType
    f32 = dt.float32
    f32r = dt.float32r
    bf16 = dt.bfloat16

    nc = bacc.Bacc("TRN2", target_bir_lowering=False, debug=False)

    L = SEQ_LEN
    C = HIDDEN
    xT = nc.dram_tensor("xT", [IN_DIM + 1, G * L], f32r, kind="ExternalInput").ap()
    wpT = nc.dram_tensor("wpT", [IN_DIM + 1, C], f32r, kind="ExternalInput").ap()
    eyeb = nc.dram_tensor("eyeb", [128, 128], bf16, kind="ExternalInput").ap()
    invC = nc.dram_tensor("invC", [128, 1], bf16, kind="ExternalInput").ap()
    invL = nc.dram_tensor("invL", [128, 1], bf16, kind="ExternalInput").ap()
    qrep = nc.dram_tensor("qrep", [DEPTH * C, L], f32, kind="ExternalInput").ap()
    diag = nc.dram_tensor("diag", [DEPTH * C, C], f32r, kind="ExternalInput").ap()
    nek = nc.dram_tensor("nek", [DEPTH * C, 1], f32, kind="ExternalInput").ap()
    waT = nc.dram_tensor("waT", [DEPTH * C, C], f32r, kind="ExternalInput").ap()
    wgT = nc.dram_tensor("wgT", [DEPTH * C, C], f32r, kind="ExternalInput").ap()
    woT = nc.dram_tensor("woT", [DEPTH * C, C], bf16, kind="ExternalInput").ap()
    if use_bias:
        ba = nc.dram_tensor("ba", [DEPTH * C, 1], f32, kind="ExternalInput").ap()
        bg = nc.dram_tensor("bg", [DEPTH * C, 1], f32, kind="ExternalInput").ap()
        bor = nc.dram_tensor("bor", [DEPTH * C, 1], f32, kind="ExternalInput").ap()
        onesr = nc.dram_tensor("onesr", [1, L], f32r, kind="ExternalInput").ap()
    if use_affine:
        gaT = nc.dram_tensor("gaT", [DEPTH * 128, L], f32, kind="ExternalInput").ap()
        beT = nc.dram_tensor("beT", [DEPTH * 128, L], f32, kind="ExternalInput").ap()
    out = nc.dram_tensor("out", [G, C], f32, kind="ExternalOutput").ap()

    with tile.TileContext(nc) as tc, ExitStack() as ctx:
        consts = ctx.enter_context(tc.tile_pool(name="consts", bufs=1))
        xpool = ctx.enter_context(tc.tile_pool(name="xpool", bufs=1))
        spool = ctx.enter_context(tc.tile_pool(name="spool", bufs=4))
        sbp = ctx.enter_context(tc.tile_pool(name="sbp", bufs=3))
        small = ctx.enter_context(tc.tile_pool(name="small", bufs=3))
        zinp = ctx.enter_context(tc.tile_pool(name="zinp", bufs=2, space="PSUM"))
        pap = ctx.enter_context(tc.tile_pool(name="pap", bufs=1, space="PSUM"))
        pgp = ctx.enter_context(tc.tile_pool(name="pgp", bufs=1, space="PSUM"))
        pzp = ctx.enter_context(tc.tile_pool(name="pzp", bufs=2, space="PSUM"))
        stp = ctx.enter_context(tc.tile_pool(name="stp", bufs=1, space="PSUM"))

        def cload(name, dram_ap, shape, dtype=f32):
            t = consts.tile(shape, dtype, tag=name)
            nc.sync.dma_start(t[:], dram_ap)
            return t

        c_wpT = cload("c_wpT", wpT, [IN_DIM + 1, C], f32r)
        c_eyeb = cload("c_eyeb", eyeb, [128, 128], bf16)
        c_invC = cload("c_invC", invC, [128, 1], bf16)
        c_invL = cload("c_invL", invL, [128, 1], bf16)
        c_eps = consts.tile([128, 1], f32, tag="c_eps")
        nc.gpsimd.memset(c_eps[:], LN_EPS)
        c_q, c_diag, c_nek, c_waT, c_wgT, c_woT = [], [], [], [], [], []
        c_ba, c_bg, c_bor, c_gaT, c_beT = [], [], [], [], []
        for d in range(DEPTH):
            rows = slice(d * C, (d + 1) * C)
            c_q.append(cload(f"c_q{d}", qrep[rows, :], [128, L]))
            c_diag.append(cload(f"c_diag{d}", diag[rows, :], [128, C], f32r))
            c_nek.append(cload(f"c_nek{d}", nek[rows, :], [128, 1]))
            c_waT.append(cload(f"c_waT{d}", waT[rows, :], [128, C], f32r))
            c_wgT.append(cload(f"c_wgT{d}", wgT[rows, :], [128, C], f32r))
            c_woT.append(cload(f"c_woT{d}", woT[rows, :], [128, C], bf16))
            if use_bias:
                c_ba.append(cload(f"c_ba{d}", ba[rows, :], [128, 1]))
                c_bg.append(cload(f"c_bg{d}", bg[rows, :], [128, 1]))
                c_bor.append(cload(f"c_bor{d}", bor[rows, :], [128, 1]))
            if use_affine:
                c_gaT.append(cload(f"c_gaT{d}", gaT[rows, :], [128, L]))
                c_beT.append(cload(f"c_beT{d}", beT[rows, :], [128, L]))
        if use_bias:
            c_onesr = cload("c_onesr", onesr, [1, L], f32r)

        for rep in range(reps):
            graphs_per_xtile = min(4, G)
            n_xtiles = (G + graphs_per_xtile - 1) // graphs_per_xtile
            xtiles = []
            for i in range(n_xtiles):
                g0 = i * graphs_per_xtile
                g1 = min(G, g0 + graphs_per_xtile)
                t = xpool.tile([IN_DIM + 1, (g1 - g0) * L], f32r,
                               tag=f"xt{i}")
                nc.scalar.dma_start(t[:], xT[:, g0 * L : g1 * L])
                xtiles.append((t, g0))

            state = [dict() for _ in range(G)]
            from collections import deque

            # WAR rings for buffers written/read by the XBAR DMA transposes,
            # which the tile dependency tracker does not model.
            ring_z2 = deque()   # t_fwd insts that read a z2 buffer
            ring_zTs = deque()  # apply insts that read a zTs buffer
            ring_zn = deque()   # t_back insts that read a zn buffer
            ring_zin = deque()  # scan insts that read a zinS buffer
            SBUFS = 3  # sbp pool depth

            def stage_A(g, d):
                st = state[g]
                if d == 0:
                    zin = zinp.tile([128, L], f32, tag="zin0")
                    xt, g0 = xtiles[g // graphs_per_xtile]
                    xg = xt[:, (g - g0) * L : (g - g0 + 1) * L]
                    nc.tensor.matmul(zin[:], c_wpT[:], xg, start=True,
                                     stop=True)
                    zin_ap = zin[:]
                else:
                    zin_ap = st["zin"][:]
                s = spool.tile([128, L], f32r, tag="s")
                i_scan = nc.vector.tensor_tensor_scan(
                    s[:], c_q[d][:], zin_ap, 0.0, Alu.mult, Alu.add
                )
                if d > 0:
                    add_dep_helper(i_scan.ins, st.pop("tback").ins,
                                   reason="scan after back-transpose")
                    ring_zin.append(i_scan)
                nc.vector.scalar_tensor_tensor(
                    s[:, KLEN:L], s[:, 0:KLEN], c_nek[d][:], s[:, KLEN:L],
                    Alu.mult, Alu.add,
                )
                pa = pap.tile([128, L], f32, tag="pa")
                nc.tensor.matmul(pa[:], c_waT[d][:], s[:], start=True,
                                 stop=True)
                pg = pgp.tile([128, L], f32, tag="pg")
                nc.tensor.matmul(pg[:], c_wgT[d][:], s[:], start=True,
                                 stop=True)
                sig = sbp.tile([128, L], bf16, tag="sig")
                bias_g = c_bg[d][:] if use_bias else 0.0
                nc.scalar.activation(sig[:], pg[:], Act.Sigmoid, bias=bias_g)
                ab = sbp.tile([128, L], bf16, tag="ab")
                if use_bias:
                    nc.scalar.activation(ab[:, 0:256], pa[:, 0:256],
                                         Act.Identity, bias=c_ba[d][:])
                    nc.scalar.activation(ab[:, 256:L], pa[:, 256:L],
                                         Act.Identity, bias=c_ba[d][:])
                else:
                    nc.scalar.copy(ab[:, 0:256], pa[:, 0:256])
                    nc.vector.tensor_copy(ab[:, 256:L], pa[:, 256:L])
                glu = sbp.tile([128, L], bf16, tag="glu")
                nc.gpsimd.tensor_mul(glu[:], ab[:], sig[:])
                st["s"], st["glu"] = s, glu

            def stage_B(g, d):
                st = state[g]
                pz = pzp.tile([128, L], f32, tag="pz")
                nc.tensor.matmul(pz[:], c_woT[d][:], st["glu"][:],
                                 start=True, stop=False)
                if use_bias:
                    nc.tensor.matmul(pz[:], c_bor[d][:], c_onesr[:],
                                     start=False, stop=False)
                nc.tensor.matmul(pz[:], c_diag[d][:], st["s"][:],
                                 start=False, stop=True)
                z2 = sbp.tile([128, L], bf16, tag="z2")
                half = L // 2
                i_lo = nc.scalar.copy(z2[:, 0:half], pz[:, 0:half])
                i_hi = nc.vector.tensor_copy(z2[:, half:L], pz[:, half:L])
                z2sq = sbp.tile([128, L], bf16, tag="z2sq")
                nc.gpsimd.tensor_mul(z2sq[:], z2[:], z2[:])
                zTs = sbp.tile([128, NCHUNK * 128], bf16, tag="zTs")
                t_fwd = nc.sync.dma_start_transpose(
                    zTs[:].rearrange("p (e f) -> p e f", f=128), z2[:]
                )
                add_dep_helper(t_fwd.ins, i_lo.ins, reason="xpose after z2 lo")
                add_dep_helper(t_fwd.ins, i_hi.ins, reason="xpose after z2 hi")
                # completion proxy: tiny tracked DMA behind t_fwd on the same
                # FIFO queue; waiting on it implies the transpose finished
                mkf = small.tile([1, 2], bf16, tag="mkf")
                p_fwd = nc.sync.dma_start(mkf[:], zTs[0:1, 0:2])
                if len(ring_z2) >= SBUFS:
                    # WAR: this z2 buffer was read by an old t_fwd
                    old = ring_z2.popleft()
                    add_dep_helper(i_lo.ins, old.ins, reason="z2 WAR")
                ring_z2.append(p_fwd)
                # WAR: this zTs buffer may still be read by old apply ops
                if len(ring_zTs) >= SBUFS:
                    for o in ring_zTs.popleft():
                        add_dep_helper(t_fwd.ins, o.ins, reason="zTs WAR")
                st["z2"], st["z2sq"], st["zTs"], st["tfwd"] = (
                    z2, z2sq, zTs, p_fwd
                )

            def stage_C(g, d):
                st = state[g]
                z2, z2sq, zTs = st["z2"], st["z2sq"], st["zTs"]
                stat = stp.tile([128, 2 * NCHUNK], f32, tag="stat")
                for j in range(NCHUNK):
                    cj = slice(j * 128, (j + 1) * 128)
                    nc.tensor.matmul(stat[:, j : j + 1], z2[:, cj],
                                     c_invC[:], start=True, stop=True)
                    nc.tensor.matmul(stat[:, NCHUNK + j : NCHUNK + j + 1],
                                     z2sq[:, cj], c_invC[:], start=True,
                                     stop=True)
                stS = small.tile([128, 2 * NCHUNK], f32, tag="stS")
                nc.vector.tensor_copy(stS[:], stat[:])
                meanS = stS[:, 0:NCHUNK]
                msqS = stS[:, NCHUNK : 2 * NCHUNK]
                m2 = small.tile([128, NCHUNK], f32, tag="m2")
                nc.gpsimd.tensor_mul(m2[:], meanS, meanS)
                varm = small.tile([128, NCHUNK], f32, tag="varm")
                nc.gpsimd.tensor_sub(varm[:], msqS, m2[:])
                sd = small.tile([128, NCHUNK], f32, tag="sd")
                nc.scalar.activation(sd[:], varm[:], Act.Sqrt, bias=c_eps[:])
                istd = small.tile([128, NCHUNK], f32, tag="istd")
                nc.vector.reciprocal(istd[:], sd[:])
                zn = sbp.tile([128, NCHUNK * 128], bf16, tag="zn")
                zTs3 = zTs[:].rearrange("p (e f) -> p e f", f=128)
                zn3 = zn[:].rearrange("p (e f) -> p e f", f=128)
                mean3 = meanS.rearrange("p (e o) -> p e o", o=1)
                istd3 = istd[:].rearrange("p (e o) -> p e o", o=1)
                meanB, _ = broadcast_tensor_aps(mean3, zTs3)
                istdB, _ = broadcast_tensor_aps(istd3, zTs3)
                i_sub = nc.gpsimd.tensor_sub(zn3, zTs3, meanB)
                add_dep_helper(i_sub.ins, st.pop("tfwd").ins,
                               reason="apply after fwd-transpose")
                # WAR: this zn buffer may still be read by an old t_back
                if len(ring_zn) >= SBUFS:
                    add_dep_helper(i_sub.ins, ring_zn.popleft().ins,
                                   reason="zn WAR vs old back-xpose")
                i_mul = nc.gpsimd.tensor_mul(zn3, zn3, istdB)
                ring_zTs.append((i_sub, i_mul))
                if use_affine:
                    zn2 = sbp.tile([128, L], bf16, tag="zn2")
                    nc.gpsimd.tensor_mul(zn2[:], zn[:], c_gaT[d][:])
                    nc.gpsimd.tensor_add(zn2[:], zn2[:], c_beT[d][:])
                    zn = zn2
                st["zn"] = zn
                st["apply_insts"] = (i_sub, i_mul)

            def stage_D(g, d):
                st = state[g]
                zn = st["zn"]
                if d < DEPTH - 1:
                    zin = sbp.tile([128, L], bf16, tag="zinS")
                    t_back = nc.sync.dma_start_transpose(
                        zin[:].rearrange("p (e f) -> p e f", f=128), zn[:]
                    )
                    for o in st.pop("apply_insts"):
                        add_dep_helper(t_back.ins, o.ins,
                                       reason="back-xpose after apply")
                    mkb = small.tile([1, 2], bf16, tag="mkb")
                    p_back = nc.sync.dma_start(mkb[:], zin[0:1, 0:2])
                    ring_zn.append(p_back)
                    if len(ring_zin) >= SBUFS:
                        old_scan = ring_zin.popleft()
                        add_dep_helper(t_back.ins, old_scan.ins,
                                       reason="zinS WAR vs old scan")
                    st["zin"] = zin
                    st["tback"] = p_back
                else:
                    st.pop("apply_insts", None)
                    po = stp.tile([1, C], f32, tag="po")
                    for j in range(NCHUNK):
                        nc.tensor.matmul(
                            po[:], c_invL[:],
                            zn[:, j * 128 : (j + 1) * 128],
                            start=(j == 0), stop=(j == NCHUNK - 1),
                        )
                    og = small.tile([1, C], f32, tag="og")
                    nc.scalar.copy(og[:], po[:])
                    nc.scalar.dma_start(out[g : g + 1, :], og[:])

            STAGES = [stage_A, stage_B, stage_C, stage_D]
            NST = DEPTH * 4
            for t in range(NST + G - 1):
                for g in range(max(0, t - NST + 1), min(G, t + 1)):
                    d, si = divmod(t - g, 4)
                    STAGES[si](g, d)

    nc.compile()
    return nc


def _host_prep(x, W_proj, b_proj, log_tau, W_in, b_in, W_out, b_out, gamma,
               beta, use_bias, use_affine):
    import ml_dtypes

    f32 = np.float32
    bf16 = ml_dtypes.bfloat16
    C = HIDDEN
    tau = np.maximum(np.exp(log_tau.astype(np.float64)), 0.001)  # (D, C)
    t = np.arange(KLEN, dtype=np.float64)
    k = np.exp(-t[None, None, :] / tau[:, :, None])  # (D, C, K)
    kn = k / (k.sum(-1)[:, :, None] + 1e-8)
    W0 = kn[:, :, KLEN - 1]  # (D, C)
    q = np.exp(1.0 / tau)
    eK = np.exp(KLEN / tau)

    qrep = np.repeat(q[:, :, None], SEQ_LEN, axis=2).reshape(
        DEPTH * C, SEQ_LEN
    )
    diag = np.zeros((DEPTH * C, C), np.float64)
    for d in range(DEPTH):
        diag[d * C : (d + 1) * C, :] = np.diag(W0[d])
    waT = np.concatenate(
        [(W_in[d, :C, :] * W0[d][None, :]).T for d in range(DEPTH)], 0
    )
    wgT = np.concatenate(
        [(W_in[d, C:, :] * W0[d][None, :]).T for d in range(DEPTH)], 0
    )
    woT = np.concatenate([W_out[d].T for d in range(DEPTH)], 0)
    wpT = np.concatenate([W_proj.T, b_proj[None, :]], 0)  # (65, C)

    common = {
        "wpT": np.ascontiguousarray(wpT, f32),
        "eyeb": np.eye(128, dtype=bf16),
        "invC": np.full((128, 1), 1.0 / HIDDEN, bf16),
        "invL": np.full((128, 1), 1.0 / SEQ_LEN, bf16),
        "qrep": np.ascontiguousarray(qrep, f32),
        "diag": np.ascontiguousarray(diag, f32),
        "nek": np.ascontiguousarray((-eK).reshape(DEPTH * C, 1), f32),
        "waT": np.ascontiguousarray(waT, f32),
        "wgT": np.ascontiguousarray(wgT, f32),
        "woT": np.ascontiguousarray(woT.astype(bf16)),
    }
    if use_bias:
        common["ba"] = np.ascontiguousarray(b_in[:, :C].reshape(-1, 1), f32)
        common["bg"] = np.ascontiguousarray(b_in[:, C:].reshape(-1, 1), f32)
        common["bor"] = np.ascontiguousarray(b_out.reshape(-1, 1), f32)
        common["onesr"] = np.ones((1, SEQ_LEN), f32)
    if use_affine:
        # zn layout is (L,C)-chunked: free index = e*128 + c -> per-channel
        # gamma/beta tile along free, same for every partition
        common["gaT"] = np.ascontiguousarray(np.concatenate(
            [np.tile(gamma[d], (128, NCHUNK)) for d in range(DEPTH)], 0
        ), f32)
        common["beT"] = np.ascontiguousarray(np.concatenate(
            [np.tile(beta[d], (128, NCHUNK)) for d in range(DEPTH)], 0
        ), f32)

    xTfull = np.concatenate([x.T, np.ones((1, x.shape[0]), x.dtype)], 0)
    in_maps = []
    per = G_PER_CORE * SEQ_LEN
    for c in range(N_CORES):
        m = dict(common)
        m["xT"] = np.ascontiguousarray(xTfull[:, c * per : (c + 1) * per], f32)
        in_maps.append(m)
    return in_maps


def prepare(x, batch, W_proj, b_proj, log_tau, W_in, b_in, W_out, b_out,
            gamma, beta, reps=1, **_ignored):
    """Build (cached) program + per-core input maps."""
    x = np.asarray(x)
    W_proj = np.asarray(W_proj)
    b_proj = np.asarray(b_proj)
    log_tau = np.asarray(log_tau)
    W_in = np.asarray(W_in)
    b_in = np.asarray(b_in)
    W_out = np.asarray(W_out)
    b_out = np.asarray(b_out)
    gamma = np.asarray(gamma)
    beta = np.asarray(beta)

    use_bias = bool(np.any(b_in != 0) or np.any(b_out != 0))
    use_affine = bool(np.any(gamma != 1) or np.any(beta != 0))

    key = (G_PER_CORE, use_bias, use_affine, reps, APPLY_MODE)
    if key not in _program_cache:
        _program_cache[key] = _build_program(
            G_PER_CORE, use_bias, use_affine, reps=reps,
            apply_mode=APPLY_MODE,
        )
    nc = _program_cache[key]

    in_maps = _host_prep(
        x, W_proj, b_proj, log_tau, W_in, b_in, W_out, b_out, gamma, beta,
        use_bias, use_affine,
    )
    return {
        "nc": nc,
        "in_maps": in_maps,
        "n_cores": N_CORES,
        "postprocess": lambda out_full: np.asarray(out_full, np.float32),
    }


def build_for_bench(reps, **inputs):
    return prepare(reps=reps, **inputs)["nc"]


def kernel(x, batch, W_proj, b_proj, log_tau, W_in, b_in, W_out, b_out,
           gamma, beta, **_ignored):
    from concourse.bass_utils import run_bass_kernel_spmd

    prep = prepare(
        x, batch, W_proj, b_proj, log_tau, W_in, b_in, W_out, b_out,
        gamma, beta,
    )
    res = run_bass_kernel_spmd(
        prep["nc"], prep["in_maps"], core_ids=list(range(prep["n_cores"]))
    )
    outs = [res.results[c]["out"] for c in range(prep["n_cores"])]
    return prep["postprocess"](np.concatenate(outs, 0))


# revision 46
# speedup vs baseline: 2.2508x; 2.2508x over previous
"""LSSEncoder Trainium2 kernel (v3).

Full inputs in, full outputs out. Shards the 128 graphs over 8 NeuronCores
(16 graphs per core), data-parallel, no collectives.

Math (matching reference.py):
  - in_proj: h = x @ W_proj.T + b_proj -> (B, C, L), L=512, C=128.
  - depthwise causal conv, kernel k[t]=exp(-t/tau) normalized; via scan
        s[l] = q*s[l-1] + z[l],  q = exp(1/tau)   (DVE, fp32 state)
        s[l>=K] -= exp(K/tau)*s[l-K]              (windowing correction)
        y = W0 * s   with W0 = kn[K-1]
    W0 is folded into the gate weights; the residual takes W0 via a
    diag(W0) matmul accumulated into the W_out PSUM tile.
  - GLU: a*sigmoid(g) @ W_out.T; residual z2 = pz + diag(W0) @ s.
  - LayerNorm over channels:
      * transpose z2 (C,L)->(L,C) with ONE SBUF->SBUF XBAR DMA transpose
      * mean/E[z^2] per position via 128x1 PE matmuls (stationary z2 /
        z2^2 chunks x ones/128 column)
      * normalize on Pool with stride-0 broadcast APs (fallback: per-chunk
        Act/DVE tensor_scalar)
  - output: mean over L via ones/512-matmul.

Schedule: 4-stage software pipeline per (graph, depth) chain, wavefront
across graphs with lag 1 so every engine sees independent work from
several graphs at once.
"""

import numpy as np

N_GRAPHS = 128
SEQ_LEN = 512
IN_DIM = 64
HIDDEN = 128
DEPTH = 3
KLEN = 256
LN_EPS = 1e-5
N_CORES = 8
G_PER_CORE = N_GRAPHS // N_CORES  # 16
NCHUNK = SEQ_LEN // 128  # 4

APPLY_MODE = "chunk"  # applies read PSUM -> Act/DVE only

_program_cache = {}


def _build_program(G, use_bias, use_affine, reps=1, apply_mode=APPLY_MODE):
    import concourse.bass as bass
    import concourse.bacc as bacc
    import concourse.tile as tile
    import concourse.mybir as mybir
    from concourse.bass import broadcast_tensor_aps
    from contextlib import ExitStack

    dt = mybir.dt
    Alu = mybir.AluOpType
    Act = mybir.ActivationFunctionType
    f32 = dt.float32
    f32r = dt.float32r
    bf16 = dt.bfloat16

    nc = bacc.Bacc("TRN2", target_bir_lowering=False, debug=False)

    L = SEQ_LEN
    C = HIDDEN
    xT = nc.dram_tensor("xT", [IN_DIM + 1, G * L], f32r, kind="ExternalInput").ap()
    wpT = nc.dram_tensor("wpT", [IN_DIM + 1, C], f32r, kind="ExternalInput").ap()
    eyeb = nc.dram_tensor("eyeb", [128, 128], bf16, kind="ExternalInput").ap()
    eyem = nc.dram_tensor("eyem", [128, 129], bf16, kind="ExternalInput").ap()
    invC = nc.dram_tensor("invC", [128, 1], bf16, kind="ExternalInput").ap()
    invL = nc.dram_tensor("invL", [128, 1], bf16, kind="ExternalInput").ap()
    qrep = nc.dram_tensor("qrep", [DEPTH * C, L], f32, kind="ExternalInput").ap()
    diag = nc.dram_tensor("diag", [DEPTH * C, C], f32r, kind="ExternalInput").ap()
    nek = nc.dram_tensor("nek", [DEPTH * C, 1], f32, kind="ExternalInput").ap()
    waT = nc.dram_tensor("waT", [DEPTH * C, C], f32r, kind="ExternalInput").ap()
    wgT = nc.dram_tensor("wgT", [DEPTH * C, C], f32r, kind="ExternalInput").ap()
    woT = nc.dram_tensor("woT", [DEPTH * C, C], bf16, kind="ExternalInput").ap()
    if use_bias:
        ba = nc.dram_tensor("ba", [DEPTH * C, 1], f32, kind="ExternalInput").ap()
        bg = nc.dram_tensor("bg", [DEPTH * C, 1], f32, kind="ExternalInput").ap()
        bor = nc.dram_tensor("bor", [DEPTH * C, 1], f32, kind="ExternalInput").ap()
        onesr = nc.dram_tensor("onesr", [1, L], f32r, kind="ExternalInput").ap()
    if use_affine:
        gaT = nc.dram_tensor("gaT", [DEPTH * 128, L], f32, kind="ExternalInput").ap()
        beT = nc.dram_tensor("beT", [DEPTH * 128, L], f32, kind="ExternalInput").ap()
    out = nc.dram_tensor("out", [G, C], f32, kind="ExternalOutput").ap()

    with tile.TileContext(nc) as tc, ExitStack() as ctx:
        consts = ctx.enter_context(tc.tile_pool(name="consts", bufs=1))
        xpool = ctx.enter_context(tc.tile_pool(name="xpool", bufs=1))
        spool = ctx.enter_context(tc.tile_pool(name="spool", bufs=4))
        sbp = ctx.enter_context(tc.tile_pool(name="sbp", bufs=3))
        small = ctx.enter_context(tc.tile_pool(name="small", bufs=3))
        zinp = ctx.enter_context(tc.tile_pool(name="zinp", bufs=1, space="PSUM"))
        pap = ctx.enter_context(tc.tile_pool(name="pap", bufs=1, space="PSUM"))
        pgp = ctx.enter_context(tc.tile_pool(name="pgp", bufs=1, space="PSUM"))
        pzp = ctx.enter_context(tc.tile_pool(name="pzp", bufs=1, space="PSUM"))
        pTp = ctx.enter_context(tc.tile_pool(name="pTp", bufs=1, space="PSUM"))
        stp = ctx.enter_context(tc.tile_pool(name="stp", bufs=1, space="PSUM"))

        def cload(name, dram_ap, shape, dtype=f32):
            t = consts.tile(shape, dtype, tag=name)
            nc.sync.dma_start(t[:], dram_ap)
            return t

        c_wpT = cload("c_wpT", wpT, [IN_DIM + 1, C], f32r)
        c_eyeb = cload("c_eyeb", eyeb, [128, 128], bf16)
        c_eyem = cload("c_eyem", eyem, [128, 129], bf16)
        c_invC = cload("c_invC", invC, [128, 1], bf16)
        c_invL = cload("c_invL", invL, [128, 1], bf16)
        c_eps = consts.tile([128, 1], f32, tag="c_eps")
        nc.gpsimd.memset(c_eps[:], LN_EPS)
        c_q, c_diag, c_nek, c_waT, c_wgT, c_woT = [], [], [], [], [], []
        c_ba, c_bg, c_bor, c_gaT, c_beT = [], [], [], [], []
        for d in range(DEPTH):
            rows = slice(d * C, (d + 1) * C)
            c_q.append(cload(f"c_q{d}", qrep[rows, :], [128, L]))
            c_diag.append(cload(f"c_diag{d}", diag[rows, :], [128, C], f32r))
            c_nek.append(cload(f"c_nek{d}", nek[rows, :], [128, 1]))
            c_waT.append(cload(f"c_waT{d}", waT[rows, :], [128, C], f32r))
            c_wgT.append(cload(f"c_wgT{d}", wgT[rows, :], [128, C], f32r))
            c_woT.append(cload(f"c_woT{d}", woT[rows, :], [128, C], bf16))
            if use_bias:
                c_ba.append(cload(f"c_ba{d}", ba[rows, :], [128, 1]))
                c_bg.append(cload(f"c_bg{d}", bg[rows, :], [128, 1]))
                c_bor.append(cload(f"c_bor{d}", bor[rows, :], [128, 1]))
            if use_affine:
                c_gaT.append(cload(f"c_gaT{d}", gaT[rows, :], [128, L]))
                c_beT.append(cload(f"c_beT{d}", beT[rows, :], [128, L]))
        if use_bias:
            c_onesr = cload("c_onesr", onesr, [1, L], f32r)

        for rep in range(reps):
            graphs_per_xtile = min(4, G)
            n_xtiles = (G + graphs_per_xtile - 1) // graphs_per_xtile
            xtiles = []
            for i in range(n_xtiles):
                g0 = i * graphs_per_xtile
                g1 = min(G, g0 + graphs_per_xtile)
                t = xpool.tile([IN_DIM + 1, (g1 - g0) * L], f32r,
                               tag=f"xt{i}")
                nc.sync.dma_start(t[:], xT[:, g0 * L : g1 * L])
                xtiles.append((t, g0))

            state = [dict() for _ in range(G)]

            def stage_A(g, d):
                st = state[g]
                if d == 0:
                    zin = zinp.tile([128, L], f32, tag="zin0")
                    xt, g0 = xtiles[g // graphs_per_xtile]
                    xg = xt[:, (g - g0) * L : (g - g0 + 1) * L]
                    nc.tensor.matmul(zin[:], c_wpT[:], xg, start=True,
                                     stop=True)
                    zin_ap = zin[:]
                else:
                    zin_ap = st["zin"][:]
                s = spool.tile([128, L], f32r, tag="s")
                nc.vector.tensor_tensor_scan(
                    s[:], c_q[d][:], zin_ap, 0.0, Alu.mult, Alu.add
                )
                nc.vector.scalar_tensor_tensor(
                    s[:, KLEN:L], s[:, 0:KLEN], c_nek[d][:], s[:, KLEN:L],
                    Alu.mult, Alu.add,
                )
                pa = pap.tile([128, L], f32, tag="pa")
                nc.tensor.matmul(pa[:], c_waT[d][:], s[:], start=True,
                                 stop=True)
                pg = pgp.tile([128, L], f32, tag="pg")
                nc.tensor.matmul(pg[:], c_wgT[d][:], s[:], start=True,
                                 stop=True)
                sig = sbp.tile([128, L], bf16, tag="sig")
                bias_g = c_bg[d][:] if use_bias else 0.0
                nc.scalar.activation(sig[:], pg[:], Act.Sigmoid, bias=bias_g)
                ab = sbp.tile([128, L], bf16, tag="ab")
                if use_bias:
                    nc.scalar.activation(ab[:], pa[:], Act.Identity,
                                         bias=c_ba[d][:])
                else:
                    nc.scalar.copy(ab[:], pa[:])
                glu = sbp.tile([128, L], bf16, tag="glu")
                nc.gpsimd.tensor_mul(glu[:], ab[:], sig[:])
                st["s"], st["glu"] = s, glu

            def stage_B(g, d):
                st = state[g]
                pz = pzp.tile([128, L], f32, tag="pz")
                nc.tensor.matmul(pz[:], c_woT[d][:], st["glu"][:],
                                 start=True, stop=False)
                if use_bias:
                    nc.tensor.matmul(pz[:], c_bor[d][:], c_onesr[:],
                                     start=False, stop=False)
                nc.tensor.matmul(pz[:], c_diag[d][:], st["s"][:],
                                 start=False, stop=True)
                z2 = sbp.tile([128, L], bf16, tag="z2")
                half = L // 2
                nc.scalar.copy(z2[:, 0:half], pz[:, 0:half])
                nc.vector.tensor_copy(z2[:, half:L], pz[:, half:L])
                z2sq = sbp.tile([128, L], bf16, tag="z2sq")
                nc.gpsimd.tensor_mul(z2sq[:], z2[:], z2[:])
                st["z2"], st["z2sq"] = z2, z2sq

            def stage_C(g, d):
                st = state[g]
                z2, z2sq = st["z2"], st["z2sq"]
                # fused transpose + stats per 128-col chunk j:
                #   z2chunk^T @ [ones/C | I] -> [mean | transpose] (130 cols
                #   with E[z^2] appended from a second mini-matmul)
                halves = []
                for h, tag in ((0, "zTa"), (1, "zTb")):
                    zt = pTp.tile([128, 2 * 130], f32, tag=tag)
                    zt3 = zt[:].rearrange("p (e f) -> p e f", f=130)
                    for jj in range(2):
                        j = 2 * h + jj
                        cj = slice(j * 128, (j + 1) * 128)
                        nc.tensor.matmul(zt3[:, jj, 0:129], z2[:, cj],
                                         c_eyem[:], start=True, stop=True)
                        nc.tensor.matmul(zt3[:, jj, 129:130], z2sq[:, cj],
                                         c_invC[:], start=True, stop=True)
                    halves.append(zt3)
                # gather (mean, msq) pairs into SBUF: cols {0, 129} of each
                # chunk -> strided AP with dims [(chunk:130), (col:129)]
                stS = small.tile([128, NCHUNK * 2], f32, tag="stS")
                stS3 = stS[:].rearrange("p (e o) -> p e o", o=2)
                for h in range(2):
                    base = halves[h][:, 0, :]
                    sel = bass.AP(base.tensor, base.offset,
                                  [base.ap[0], [130, 2], [129, 2]])
                    nc.vector.tensor_copy(stS3[:, 2 * h : 2 * h + 2, :], sel)
                meanS = stS3[:, :, 0:1]
                msqS = stS3[:, :, 1:2]
                m2 = small.tile([128, NCHUNK], f32, tag="m2")
                m23 = m2[:].rearrange("p (e o) -> p e o", o=1)
                nc.gpsimd.tensor_mul(m23, meanS, meanS)
                varm = small.tile([128, NCHUNK], f32, tag="varm")
                varm3 = varm[:].rearrange("p (e o) -> p e o", o=1)
                nc.gpsimd.tensor_sub(varm3, msqS, m23)
                sd = small.tile([128, NCHUNK], f32, tag="sd")
                nc.scalar.activation(sd[:], varm[:], Act.Sqrt, bias=c_eps[:])
                istd = small.tile([128, NCHUNK], f32, tag="istd")
                nc.vector.reciprocal(istd[:], sd[:])
                istd3 = istd[:].rearrange("p (e o) -> p e o", o=1)
                mscP = small.tile([128, NCHUNK], f32, tag="mscP")
                mscP3 = mscP[:].rearrange("p (e o) -> p e o", o=1)
                nc.gpsimd.tensor_mul(mscP3, meanS, istd3)
                # move the transposed data to SBUF so Pool can normalize it
                zTs = sbp.tile([128, NCHUNK * 128], bf16, tag="zTs")
                zTs3 = zTs[:].rearrange("p (e f) -> p e f", f=128)
                nc.scalar.copy(zTs3[:, 0:2, :], halves[0][:, :, 1:129])
                nc.vector.tensor_copy(zTs3[:, 2:4, :], halves[1][:, :, 1:129])
                zn = sbp.tile([128, NCHUNK * 128], bf16, tag="zn")
                zn3 = zn[:].rearrange("p (e f) -> p e f", f=128)
                istdB, _ = broadcast_tensor_aps(istd3, zTs3)
                mscPB, _ = broadcast_tensor_aps(mscP3, zTs3)
                nc.gpsimd.tensor_mul(zn3, zTs3, istdB)
                nc.gpsimd.tensor_sub(zn3, zn3, mscPB)
                if use_affine:
                    zn2 = sbp.tile([128, L], bf16, tag="zn2")
                    nc.gpsimd.tensor_mul(zn2[:], zn[:], c_gaT[d][:])
                    nc.gpsimd.tensor_add(zn2[:], zn2[:], c_beT[d][:])
                    zn = zn2
                st["zn"] = zn

            def stage_D(g, d):
                st = state[g]
                zn = st["zn"]
                if d < DEPTH - 1:
                    zin = zinp.tile([128, L], bf16, tag="zinT")
                    for j in range(NCHUNK):
                        cj = slice(j * 128, (j + 1) * 128)
                        nc.tensor.transpose(zin[:, cj], zn[:, cj], c_eyeb[:])
                    st["zin"] = zin
                else:
                    po = stp.tile([1, C], f32, tag="po")
                    for j in range(NCHUNK):
                        nc.tensor.matmul(
                            po[:], c_invL[:],
                            zn[:, j * 128 : (j + 1) * 128],
                            start=(j == 0), stop=(j == NCHUNK - 1),
                        )
                    og = small.tile([1, C], f32, tag="og")
                    nc.scalar.copy(og[:], po[:])
                    nc.sync.dma_start(out[g : g + 1, :], og[:])

            STAGES = [stage_A, stage_B, stage_C, stage_D]
            NST = DEPTH * 4
            for t in range(NST + G - 1):
                for g in range(max(0, t - NST + 1), min(G, t + 1)):
                    d, si = divmod(t - g, 4)
                    STAGES[si](g, d)

    nc.compile()
    return nc


def _host_prep(x, W_proj, b_proj, log_tau, W_in, b_in, W_out, b_out, gamma,
               beta, use_bias, use_affine):
    import ml_dtypes

    f32 = np.float32
    bf16 = ml_dtypes.bfloat16
    C = HIDDEN
    tau = np.maximum(np.exp(log_tau.astype(np.float64)), 0.001)  # (D, C)
    t = np.arange(KLEN, dtype=np.float64)
    k = np.exp(-t[None, None, :] / tau[:, :, None])  # (D, C, K)
    kn = k / (k.sum(-1)[:, :, None] + 1e-8)
    W0 = kn[:, :, KLEN - 1]  # (D, C)
    q = np.exp(1.0 / tau)
    eK = np.exp(KLEN / tau)

    qrep = np.repeat(q[:, :, None], SEQ_LEN, axis=2).reshape(
        DEPTH * C, SEQ_LEN
    )
    diag = np.zeros((DEPTH * C, C), np.float64)
    for d in range(DEPTH):
        diag[d * C : (d + 1) * C, :] = np.diag(W0[d])
    waT = np.concatenate(
        [(W_in[d, :C, :] * W0[d][None, :]).T for d in range(DEPTH)], 0
    )
    wgT = np.concatenate(
        [(W_in[d, C:, :] * W0[d][None, :]).T for d in range(DEPTH)], 0
    )
    woT = np.concatenate([W_out[d].T for d in range(DEPTH)], 0)
    wpT = np.concatenate([W_proj.T, b_proj[None, :]], 0)  # (65, C)

    common = {
        "wpT": np.ascontiguousarray(wpT, f32),
        "eyeb": np.eye(128, dtype=bf16),
        "eyem": np.concatenate(
            [np.full((128, 1), 1.0 / HIDDEN), np.eye(128)], 1
        ).astype(bf16),
        "invC": np.full((128, 1), 1.0 / HIDDEN, bf16),
        "invL": np.full((128, 1), 1.0 / SEQ_LEN, bf16),
        "qrep": np.ascontiguousarray(qrep, f32),
        "diag": np.ascontiguousarray(diag, f32),
        "nek": np.ascontiguousarray((-eK).reshape(DEPTH * C, 1), f32),
        "waT": np.ascontiguousarray(waT, f32),
        "wgT": np.ascontiguousarray(wgT, f32),
        "woT": np.ascontiguousarray(woT.astype(bf16)),
    }
    if use_bias:
        common["ba"] = np.ascontiguousarray(b_in[:, :C].reshape(-1, 1), f32)
        common["bg"] = np.ascontiguousarray(b_in[:, C:].reshape(-1, 1), f32)
        common["bor"] = np.ascontiguousarray(b_out.reshape(-1, 1), f32)
        common["onesr"] = np.ones((1, SEQ_LEN), f32)
    if use_affine:
        # zn layout is (L,C)-chunked: free index = e*128 + c -> per-channel
        # gamma/beta tile along free, same for every partition
        common["gaT"] = np.ascontiguousarray(np.concatenate(
            [np.tile(gamma[d], (128, NCHUNK)) for d in range(DEPTH)], 0
        ), f32)
        common["beT"] = np.ascontiguousarray(np.concatenate(
            [np.tile(beta[d], (128, NCHUNK)) for d in range(DEPTH)], 0
        ), f32)

    xTfull = np.concatenate([x.T, np.ones((1, x.shape[0]), x.dtype)], 0)
    in_maps = []
    per = G_PER_CORE * SEQ_LEN
    for c in range(N_CORES):
        m = dict(common)
        m["xT"] = np.ascontiguousarray(xTfull[:, c * per : (c + 1) * per], f32)
        in_maps.append(m)
    return in_maps


def prepare(x, batch, W_proj, b_proj, log_tau, W_in, b_in, W_out, b_out,
            gamma, beta, reps=1, **_ignored):
    """Build (cached) program + per-core input maps."""
    x = np.asarray(x)
    W_proj = np.asarray(W_proj)
    b_proj = np.asarray(b_proj)
    log_tau = np.asarray(log_tau)
    W_in = np.asarray(W_in)
    b_in = np.asarray(b_in)
    W_out = np.asarray(W_out)
    b_out = np.asarray(b_out)
    gamma = np.asarray(gamma)
    beta = np.asarray(beta)

    use_bias = bool(np.any(b_in != 0) or np.any(b_out != 0))
    use_affine = bool(np.any(gamma != 1) or np.any(beta != 0))

    key = (G_PER_CORE, use_bias, use_affine, reps, APPLY_MODE)
    if key not in _program_cache:
        _program_cache[key] = _build_program(
            G_PER_CORE, use_bias, use_affine, reps=reps,
            apply_mode=APPLY_MODE,
        )
    nc = _program_cache[key]

    in_maps = _host_prep(
        x, W_proj, b_proj, log_tau, W_in, b_in, W_out, b_out, gamma, beta,
        use_bias, use_affine,
    )
    return {
        "nc": nc,
        "in_maps": in_maps,
        "n_cores": N_CORES,
        "postprocess": lambda out_full: np.asarray(out_full, np.float32),
    }


def build_for_bench(reps, **inputs):
    return prepare(reps=reps, **inputs)["nc"]


def kernel(x, batch, W_proj, b_proj, log_tau, W_in, b_in, W_out, b_out,
           gamma, beta, **_ignored):
    from concourse.bass_utils import run_bass_kernel_spmd

    prep = prepare(
        x, batch, W_proj, b_proj, log_tau, W_in, b_in, W_out, b_out,
        gamma, beta,
    )
    res = run_bass_kernel_spmd(
        prep["nc"], prep["in_maps"], core_ids=list(range(prep["n_cores"]))
    )
    outs = [res.results[c]["out"] for c in range(prep["n_cores"])]
    return prep["postprocess"](np.concatenate(outs, 0))
